# revision 23
# baseline (speedup 1.0000x reference)
"""ArSSR Trainium2 kernel: Conv3d encoder + trilinear grid_sample + 2x 4-layer MLP.

Self-contained: kernel(**inputs) -> np.ndarray, distributed over 8 NeuronCores.
Sharding: cores 0-3 handle batch 0, cores 4-7 batch 1; each core takes a
contiguous quarter (65536) of the K=262144 points of its batch item.

v3: z-minor fm layout (one 2KB gather descriptor = all 8 trilinear corners),
host-side point sort by cell id (sequential-ish HBM gather), single index set,
all-k=128 dense layers (xyz k-tiles zero-padded to 128 for FWL + uniform MMs),
chunk-outer dense loops so evacuations pipeline with matmuls, residual join
via pre-added x+h, blend mults split DVE/GPSIMD, evacuations split ACT/DVE.
"""
import os

import numpy as np
import ml_dtypes

import concourse.bass as bass
import concourse.mybir as mybir
import concourse.tile as tile
from concourse import bacc
from concourse.bass_utils import run_bass_kernel_spmd

f32 = mybir.dt.float32
bf16 = mybir.dt.bfloat16
i16 = mybir.dt.int16
AOT = mybir.AluOpType
ACTF = mybir.ActivationFunctionType

N = 2
HWD = 64
K = HWD ** 3
HL = 32
FEAT = 128
WIDTH = 256
IN_DIM = FEAT + 3

ST = 2048                        # supertile points
NST = 32
P = ST * NST                     # points per core (65536 full)
G = P // 128                     # point-major columns
STG = ST // 128                  # 16

RZ0 = 15
NZ = 17                          # y values 15..31
NZP = 18                         # z slots 15..32 (z=32 finite garbage, masked)
ROWW = 32
NVOX = NZ * ROWW * NZP           # 9792 rows/voxels, (y,x,z) order
# row j = ((y-15)*32 + x)*18 + (z-15); feature of voxel u lands at row u
# (slot0), u-18 (slot1: x+1), u-576 (slot2: y+1), u-594 (slot3: both)
QOFF = [0, 18, 576, 594]

NFREE = int(os.environ.get("ARSSR_NFREE", "512"))   # matmul free dim (<=512: psum bank)
GPS_MULTS = int(os.environ.get("ARSSR_GPSM", "1"))   # blend mults on gpsimd
GPS_XRES = os.environ.get("ARSSR_GPSXRES", "0") == "1"  # keep gpsimd FIFO = gathers only
XCM_DMA = os.environ.get("ARSSR_XCMDMA", "0") == "1"  # dma_start can't read PSUM
GCH = 1024                       # dma_gather index cap per call

bf = ml_dtypes.bfloat16


def build_core_kernel(nc: "bacc.Bacc"):
    patches_d = nc.dram_tensor("patches", [28, NVOX], bf16, kind="ExternalInput")
    w2 = nc.dram_tensor("w2", [28, 128], bf16, kind="ExternalInput")
    xyz_pm_d = nc.dram_tensor("xyz_pm", [128, G, 3], f32, kind="ExternalInput")
    xyz_wr_d = nc.dram_tensor("xyz_wr", [16, P // 16, 3], f32, kind="ExternalInput")
    xyzb = nc.dram_tensor("xyzb", [3, P], bf16, kind="ExternalInput")
    wts = {}
    # s1w0/s2w0 second k-tile host-padded to 128 rows (zeros) for FWL
    for name, (kk, mm) in [
        ("s1w0", (256, WIDTH)), ("s1w1", (WIDTH, WIDTH)), ("s1w2", (WIDTH, WIDTH)),
        ("s1w3", (WIDTH, IN_DIM)),
        ("s2w0", (256, WIDTH)), ("s2w1", (WIDTH, WIDTH)), ("s2w2", (WIDTH, WIDTH)),
        ("s2w3", (WIDTH, 1)),
    ]:
        wts[name] = nc.dram_tensor(name, [kk, mm], bf16, kind="ExternalInput")
    biases_d = nc.dram_tensor("biases", [128, 16], f32, kind="ExternalInput")
    ident_d = nc.dram_tensor("ident", [128, 128], bf16, kind="ExternalInput")
    out_d = nc.dram_tensor("out", [P], f32, kind="ExternalOutput")

    with tile.TileContext(nc) as tc:
        with (
            tc.tile_pool(name="const", bufs=1) as const,
            tc.tile_pool(name="keep", bufs=1) as keep,
            tc.tile_pool(name="dram", bufs=1, space="DRAM") as dram,
            tc.tile_pool(name="ps", bufs=4, space="PSUM") as psp,
        ):
            # ---------- constants ----------
            wtile = {}
            for name in wts:
                kk, mm = wts[name].shape
                tiles = []
                for k0 in range(0, kk, 128):
                    ksz = min(128, kk - k0)
                    t = const.tile([ksz, mm], bf16, tag=f"{name}_{k0}")
                    nc.sync.dma_start(t[:], wts[name].ap()[k0:k0 + ksz, :])
                    tiles.append(t)
                wtile[name] = tiles
            bias_sb = const.tile([128, 16], f32)
            nc.sync.dma_start(bias_sb[:], biases_d.ap())
            ident = const.tile([128, 128], bf16)
            nc.sync.dma_start(ident[:], ident_d.ap())

            fm_dram = dram.tile([NVOX + 2, 512], bf16)  # +2: z-pair AP overhang

            # whole-shard prep outputs
            w8 = [keep.tile([128, G], f32, tag=f"w8_{i}", name=f"w8_{i}")
                  for i in range(8)]
            idx_w = keep.tile([128, P // 16], i16, tag="iw", name="iw")
            # ping-pong padded xyz / residual-xyz tiles (rows 3:128 stay zero)
            kxyz = [keep.tile([128, ST], bf16, tag=f"kxyz{i}", name=f"kxyz{i}")
                    for i in range(2)]
            khh1 = [keep.tile([128, ST], bf16, tag=f"khh1{i}", name=f"khh1{i}")
                    for i in range(2)]
            for t in kxyz + khh1:
                nc.vector.memset(t[:], 0.0)

            # ---------- conv encoder ----------
            with tc.tile_pool(name="convp", bufs=1) as convp, \
                 tc.tile_pool(name="convs", bufs=2) as convs:
                patches = convp.tile([28, NVOX], bf16)
                nc.sync.dma_start(patches[:], patches_d.ap())
                w2_sb = convp.tile([28, 128], bf16)
                nc.sync.dma_start(w2_sb[:], w2.ap())

                # zero quad-slot tails no conv write covers (weight-masked on
                # access but must be finite), plus the 2 AP-overhang rows
                zrow = convp.tile([1, 4096], bf16)
                nc.vector.memset(zrow[:], 0.0)

                def zfill(row0, nrows, col0, width):
                    r = row0
                    while r < row0 + nrows:
                        cnt = min(4096 // width, row0 + nrows - r)
                        dst = bass.AP(
                            fm_dram[:].tensor,
                            fm_dram[:].offset + r * 512 + col0,
                            [[512, cnt], [1, width]])
                        nc.sync.dma_start(dst, zrow[0:1, 0:cnt * width])
                        r += cnt

                zfill(NVOX - 18, 18, 128, 128)
                zfill(NVOX - 576, 576, 256, 128)
                zfill(NVOX - 594, 594, 384, 128)
                zfill(NVOX, 2, 0, 512)

                NT = (NVOX + 127) // 128  # 77
                for t4 in range(0, NT, 2):
                    pc = psp.tile([128, 1024], f32, tag="ps")
                    cnt = min(2, NT - t4)
                    for q in range(cnt):
                        t = t4 + q
                        vsz = min(128, NVOX - t * 128)
                        nc.tensor.matmul(
                            pc[0:vsz, q * 512:q * 512 + 128],
                            patches[:, t * 128:t * 128 + vsz],
                            w2_sb[:],
                            start=True, stop=True,
                        )
                    fmsb = convs.tile([128, 2, 128], bf16, tag="fmsb")
                    for q in range(cnt):
                        vsz = min(128, NVOX - (t4 + q) * 128)
                        nc.scalar.copy(fmsb[0:vsz, q, :],
                                       pc[0:vsz, q * 512:q * 512 + 128])
                    for s in range(4):
                        off = QOFF[s]
                        for q in range(cnt):
                            t = t4 + q
                            vsz = min(128, NVOX - t * 128)
                            lo = max(0, off - t * 128)
                            if lo >= vsz:
                                continue
                            dst = bass.AP(
                                fm_dram[:].tensor,
                                fm_dram[:].offset + (t * 128 + lo - off) * 512
                                + s * 128,
                                [[512, vsz - lo], [1, 128]])
                            nc.sync.dma_start(dst, fmsb[lo:vsz, q, :])

            # ---------- whole-shard point prep ----------
            with tc.tile_pool(name="prep", bufs=1) as prep:
                xyz_pm = prep.tile([128, G, 3], f32)
                nc.sync.dma_start(xyz_pm[:], xyz_pm_d.ap())
                MAGIC = 12582912.0

                def floor_frac(col):
                    u = prep.tile([128, G], f32, tag=f"u{col}")
                    nc.vector.tensor_scalar(u[:], xyz_pm[:, :, col], 16.0, 15.5,
                                            AOT.mult, AOT.add)
                    fl = prep.tile([128, G], f32, tag=f"fl{col}")
                    gt = prep.tile([128, G], f32, tag=f"gt{col}")
                    nc.vector.tensor_scalar(fl[:], u[:], MAGIC, -MAGIC,
                                            AOT.add, AOT.add)
                    nc.vector.tensor_tensor(gt[:], fl[:], u[:], AOT.is_gt)
                    nc.vector.tensor_tensor(fl[:], fl[:], gt[:], AOT.subtract)
                    w = prep.tile([128, G], f32, tag=f"w{col}")
                    nc.vector.tensor_tensor(w[:], u[:], fl[:], AOT.subtract)
                    return fl, w

                flz, wz = floor_frac(0)
                fly, wy = floor_frac(1)
                flx, wx = floor_frac(2)

                def ab(fl, w, a_t, b_t, tagp):
                    # a = 1-w ; b = w * (fl < 31)  [mask via min(31-fl, 1)]
                    m = prep.tile([128, G], f32, tag=f"m{tagp}")
                    nc.vector.tensor_scalar(a_t[:], w[:], -1.0, 1.0,
                                            AOT.mult, AOT.add)
                    nc.vector.tensor_scalar(m[:], fl[:], -1.0, 31.0,
                                            AOT.mult, AOT.add)
                    nc.vector.tensor_scalar(m[:], m[:], 1.0, None, AOT.min)
                    nc.vector.tensor_tensor(b_t[:], w[:], m[:], AOT.mult)

                az = prep.tile([128, G], f32, tag="az")
                bz = prep.tile([128, G], f32, tag="bz")
                ay = prep.tile([128, G], f32, tag="ay")
                by = prep.tile([128, G], f32, tag="by")
                axk = prep.tile([128, G], f32, tag="axk")
                bxk = prep.tile([128, G], f32, tag="bxk")
                ab(flz, wz, az, bz, "z")
                ab(fly, wy, ay, by, "y")
                ab(flx, wx, axk, bxk, "x")
                # slot order within a row: (y,x) (y,x+1) (y+1,x) (y+1,x+1)
                wyx = []
                for i, (ty, tx) in enumerate([(ay, axk), (ay, bxk),
                                              (by, axk), (by, bxk)]):
                    t = prep.tile([128, G], f32, tag=f"wyx{i}", name=f"wyx{i}")
                    nc.vector.tensor_tensor(t[:], ty[:], tx[:], AOT.mult)
                    wyx.append(t)
                for zi, tz in enumerate([az, bz]):
                    for s in range(4):
                        nc.vector.tensor_tensor(w8[zi * 4 + s][:], tz[:], wyx[s][:],
                                                AOT.mult)

                # wrapped-16 row index: j = fly*576 + flx*18 + flz - 8655
                F = P // 16
                WCH = min(512, F)
                iw16 = prep.tile([16, F], i16, tag="iw16", name="iw16")
                for ch in range(F // WCH):
                    xw = prep.tile([16, WCH, 3], f32, tag="xw")
                    nc.sync.dma_start(xw[:],
                                      xyz_wr_d.ap()[:, ch * WCH:(ch + 1) * WCH, :])

                    def wfloor(col, tag):
                        u = prep.tile([16, WCH], f32, tag=f"wu{tag}")
                        nc.vector.tensor_scalar(u[:], xw[:, :, col], 16.0, 15.5,
                                                AOT.mult, AOT.add)
                        fl = prep.tile([16, WCH], f32, tag=f"wfl{tag}")
                        gt = prep.tile([16, WCH], f32, tag=f"wgt{tag}")
                        nc.vector.tensor_scalar(fl[:], u[:], MAGIC, -MAGIC,
                                                AOT.add, AOT.add)
                        nc.vector.tensor_tensor(gt[:], fl[:], u[:], AOT.is_gt)
                        nc.vector.tensor_tensor(fl[:], fl[:], gt[:], AOT.subtract)
                        return fl

                    wflz = wfloor(0, "z")
                    wfly = wfloor(1, "y")
                    wflx = wfloor(2, "x")
                    ja = prep.tile([16, WCH], f32, tag="ja")
                    jb = prep.tile([16, WCH], f32, tag="jb")
                    nc.vector.tensor_scalar(ja[:], wfly[:], float(ROWW * NZP),
                                            -8655.0, AOT.mult, AOT.add)
                    nc.vector.tensor_scalar(jb[:], wflx[:], float(NZP), None,
                                            AOT.mult)
                    nc.vector.tensor_tensor(ja[:], ja[:], jb[:], AOT.add)
                    nc.vector.tensor_tensor(ja[:], ja[:], wflz[:], AOT.add)
                    nc.vector.tensor_copy(
                        iw16[:, ch * WCH:(ch + 1) * WCH], ja[:])
                # replicate 16 -> 128
                for r in range(8):
                    nc.sync.dma_start(idx_w[r * 16:(r + 1) * 16, :], iw16[:])

            # ---------- supertile loop ----------
            # z-pair: a 1024-elem read at row j covers rows j (z0), j+1 (z1)
            fm_in = bass.AP(fm_dram[:].tensor, fm_dram[:].offset,
                            [[512, NVOX], [1, 1024]])

            with (
                tc.tile_pool(name="gath", bufs=2) as gath,
                tc.tile_pool(name="actp", bufs=2) as actp,
                tc.tile_pool(name="outp", bufs=2) as outp,
            ):
                for st in range(NST):
                    gt_t = gath.tile([128, STG, 1024], bf16, tag="g", name="g")
                    for ch in range(ST // GCH):
                        nc.gpsimd.dma_gather(
                            gt_t[:, ch * (GCH // 128):(ch + 1) * (GCH // 128), :],
                            fm_in,
                            idx_w[:, st * (ST // 16) + ch * (GCH // 16):
                                  st * (ST // 16) + (ch + 1) * (GCH // 16)],
                            num_idxs=GCH, num_idxs_reg=GCH,
                            elem_size=1024, elem_step=512, transpose=False,
                        )

                    # flat-8 blend: scale each slot by w8 in place, then sum.
                    sl = slice(st * STG, (st + 1) * STG)
                    for s in range(8):
                        wb = w8[s][:, sl].unsqueeze(2) \
                            .broadcast_to([128, STG, 128])
                        eng = nc.gpsimd if s >= 8 - GPS_MULTS else nc.vector
                        eng.tensor_tensor(
                            gt_t[:, :, s * 128:(s + 1) * 128],
                            gt_t[:, :, s * 128:(s + 1) * 128], wb, AOT.mult)
                    nc.vector.tensor_tensor(gt_t[:, :, 0:512],
                                            gt_t[:, :, 0:512],
                                            gt_t[:, :, 512:1024], AOT.add)
                    nc.vector.tensor_tensor(gt_t[:, :, 0:256],
                                            gt_t[:, :, 0:256],
                                            gt_t[:, :, 256:512], AOT.add)
                    nc.vector.tensor_tensor(gt_t[:, :, 0:128],
                                            gt_t[:, :, 0:128],
                                            gt_t[:, :, 128:256], AOT.add)

                    # transpose to channel-major
                    xcm = actp.tile([128, ST], bf16, tag="xcm")
                    ptile = psp.tile([128, 1024], f32, tag="ps")
                    ptb = ptile.bitcast(bf16)  # [128, 2048] bf16 view
                    for g in range(STG):
                        nc.tensor.transpose(ptb[:, g * 128:(g + 1) * 128],
                                            gt_t[:, g, 0:128], ident[:])
                    if XCM_DMA:
                        nc.sync.dma_start(xcm[:], ptb[:, 0:ST])
                    else:
                        nc.vector.tensor_copy(xcm[:], ptb[:, 0:ST])

                    xyzp = kxyz[st % 2]
                    nc.sync.dma_start(xyzp[0:3, :],
                                      xyzb.ap()[:, st * ST:(st + 1) * ST])

                    def dense(k_tiles, w_aps, bias_cols, osizes, tagp,
                              evac, evac_osizes=None, out_tiles=None):
                        """k_tiles: list of [128, ST] tiles (ksz=128).
                        Chunk-outer: (mi0,h0),(mi1,h0),(mi0,h1),(mi1,h1)."""
                        outs = [out_tiles[mi] if out_tiles and out_tiles[mi]
                                is not None else
                                actp.tile([128, ST], bf16, tag=f"h{tagp}{mi}",
                                          name=f"h{tagp}{mi}")
                                for mi in range(len(osizes))]
                        nk = len(k_tiles)
                        for half in range(2):
                            for mi, osz in enumerate(osizes):
                                pm_t = psp.tile([128, 1024], f32, tag="ps",
                                                name="pmt")
                                for ki, xt in enumerate(k_tiles):
                                    for j in range(1024 // NFREE):
                                        jj = half * (1024 // NFREE) + j
                                        nc.tensor.matmul(
                                            pm_t[0:osz, j * NFREE:(j + 1) * NFREE],
                                            w_aps[mi][ki],
                                            xt[:, jj * NFREE:(jj + 1) * NFREE],
                                            start=(ki == 0), stop=(ki == nk - 1),
                                        )
                                eosz = (evac_osizes[mi] if evac_osizes
                                        else osz)
                                bcol = bias_sb[0:eosz,
                                               bias_cols[mi]:bias_cols[mi] + 1]
                                osl = slice(half * 1024, (half + 1) * 1024)
                                if evac[mi] == "act":
                                    nc.scalar.activation(outs[mi][0:eosz, osl],
                                                         pm_t[0:eosz, :],
                                                         ACTF.Relu, bias=bcol)
                                else:
                                    nc.vector.tensor_scalar(outs[mi][0:eosz, osl],
                                                            pm_t[0:eosz, :],
                                                            bcol, 0.0, AOT.add,
                                                            AOT.max)
                        return outs

                    wa = wtile["s1w0"]
                    h = dense([xyzp, xcm],
                              [[wa[1][:, 0:128], wa[0][:, 0:128]],
                               [wa[1][:, 128:256], wa[0][:, 128:256]]],
                              [0, 1], [128, 128], "A", ("act", "act"))
                    wa = wtile["s1w1"]
                    h = dense([h[0], h[1]],
                              [[wa[0][:, 0:128], wa[1][:, 0:128]],
                               [wa[0][:, 128:256], wa[1][:, 128:256]]],
                              [2, 3], [128, 128], "B", ("act", "act"))
                    wa = wtile["s1w2"]
                    h = dense([h[0], h[1]],
                              [[wa[0][:, 0:128], wa[1][:, 0:128]],
                               [wa[0][:, 128:256], wa[1][:, 128:256]]],
                              [4, 5], [128, 128], "A", ("act", "act"))
                    wa = wtile["s1w3"]
                    hh1 = khh1[st % 2]
                    hh = dense([h[0], h[1]],
                               [[wa[0][:, 0:128], wa[1][:, 0:128]],
                                [wa[0][:, 128:131], wa[1][:, 128:131]]],
                               [6, 7], [128, 3], "D", ("act", "act"),
                               out_tiles=[None, hh1])

                    # residual join on PE: s2L0 accumulates x and h k-tiles
                    # (xyzp/hh1 rows 3:128 are zero, weights there zero too)
                    wa = wtile["s2w0"]
                    h = dense([xyzp, xcm, hh1, hh[0]],
                              [[wa[1][:, 0:128], wa[0][:, 0:128],
                                wa[1][:, 0:128], wa[0][:, 0:128]],
                               [wa[1][:, 128:256], wa[0][:, 128:256],
                                wa[1][:, 128:256], wa[0][:, 128:256]]],
                              [8, 9], [128, 128], "A", ("act", "act"))
                    wa = wtile["s2w1"]
                    h = dense([h[0], h[1]],
                              [[wa[0][:, 0:128], wa[1][:, 0:128]],
                               [wa[0][:, 128:256], wa[1][:, 128:256]]],
                              [10, 11], [128, 128], "B", ("act", "act"))
                    wa = wtile["s2w2"]
                    h = dense([h[0], h[1]],
                              [[wa[0][:, 0:128], wa[1][:, 0:128]],
                               [wa[0][:, 128:256], wa[1][:, 128:256]]],
                              [12, 13], [128, 128], "A", ("act", "act"))
                    wa = wtile["s2w3"]
                    ofinal = outp.tile([1, ST], f32, tag="of")
                    for half in range(2):
                        pm_t = psp.tile([128, 1024], f32, tag="ps", name="pmt2")
                        for ki, xt in enumerate(h):
                            for j in range(1024 // NFREE):
                                jj = half * (1024 // NFREE) + j
                                nc.tensor.matmul(
                                    pm_t[0:1, j * NFREE:(j + 1) * NFREE],
                                    wa[ki][:, 0:1],
                                    xt[:, jj * NFREE:(jj + 1) * NFREE],
                                    start=(ki == 0), stop=(ki == 1),
                                )
                        nc.scalar.activation(
                            ofinal[0:1, half * 1024:(half + 1) * 1024],
                            pm_t[0:1, :], ACTF.Relu, bias=bias_sb[0:1, 14:15])
                    nc.sync.dma_start(
                        bass.AP(out_d.ap().tensor, st * ST, [[ST, 1], [1, ST]]),
                        ofinal[:])
    return nc


_CACHED = {}


def _get_compiled():
    if "nc" not in _CACHED:
        nc = bacc.Bacc("TRN2", target_bir_lowering=False, debug=False)
        build_core_kernel(nc)
        nc.compile()
        _CACHED["nc"] = nc
    return _CACHED["nc"]


def _make_patches(img):
    """img: [32,32,32] f32 (z,y,x). Returns [28, NVOX] bf16 patches in
    (y 15..31, x 0..31, z 15..32) voxel order; row 27 = ones (bias)."""
    imgp = np.pad(img, ((1, 2), (1, 1), (1, 1)))
    out = np.zeros((28, NVOX), np.float32)
    t = 0
    for dz in range(3):
        for dy in range(3):
            for dx in range(3):
                blk = imgp[15 + dz:33 + dz, 15 + dy:32 + dy, dx:32 + dx]
                out[t] = blk.transpose(1, 2, 0).ravel()  # (y,x,z)
                t += 1
    out[27] = 1.0
    return out.astype(bf)


def _prep_in_maps(img_lr, xyz_hr, conv_w, conv_b, s1, s2):
    w2 = np.zeros((28, 128), np.float32)
    w2[:27, :] = conv_w.reshape(FEAT, 27).T
    w2[27, :] = conv_b
    w2 = w2.astype(bf)

    patches = [_make_patches(np.asarray(img_lr[b, 0], np.float32))
               for b in range(N)]

    def half(v, lo, hi):
        r = np.zeros(128, np.float32)
        r[:hi - lo] = v[lo:hi]
        return r

    rows = [
        half(s1[0][1], 0, 128), half(s1[0][1], 128, 256),
        half(s1[1][1], 0, 128), half(s1[1][1], 128, 256),
        half(s1[2][1], 0, 128), half(s1[2][1], 128, 256),
        half(s1[3][1], 0, 128), half(s1[3][1], 128, 131),
        half(s2[0][1], 0, 128), half(s2[0][1], 128, 256),
        half(s2[1][1], 0, 128), half(s2[1][1], 128, 256),
        half(s2[2][1], 0, 128), half(s2[2][1], 128, 256),
        half(s2[3][1], 0, 1), np.zeros(128, np.float32),
    ]
    biases = np.ascontiguousarray(np.stack(rows).astype(np.float32).T)

    def padk(w):
        # pad [131, m] -> [256, m] with zero rows so the xyz k-tile is k=128
        wp = np.zeros((256, w.shape[1]), np.float32)
        wp[:w.shape[0]] = w
        return wp

    wmats = {}
    for pre, params in [("s1", s1), ("s2", s2)]:
        for li, (w, b) in enumerate(params):
            w = np.asarray(w, np.float32)
            if li == 0:
                w = padk(w)
            wmats[f"{pre}w{li}"] = np.ascontiguousarray(w).astype(bf)

    in_maps = []
    perms = []
    for core in range(8):
        b, q = core // 4, core % 4
        xyz_shard = np.asarray(xyz_hr[b, q * (K // 4):q * (K // 4) + P],
                               np.float32)
        # sort points by fm row id for HBM locality in the gather
        u = np.floor(xyz_shard * 16.0 + 15.5).astype(np.int64)  # (z,y,x)
        cell = (u[:, 1] * (ROWW * NZP) + u[:, 2] * NZP + u[:, 0])
        perm = np.argsort(cell, kind="stable")
        # stripe each 1024-point gather block so DMA queue q (descriptor
        # i%16) reads a contiguous sorted run instead of interleaved cells
        order = np.arange(1024).reshape(16, 64).T.ravel()
        perm = perm.reshape(-1, 1024)[:, order].ravel()
        perms.append(perm)
        xyz_shard = xyz_shard[perm]

        xyz_pm = np.ascontiguousarray(
            xyz_shard.reshape(P // 128, 128, 3).transpose(1, 0, 2))
        xyz_wr = np.ascontiguousarray(
            xyz_shard.reshape(P // 16, 16, 3).transpose(1, 0, 2))
        xyzb_cm = np.ascontiguousarray(xyz_shard.T.astype(bf))
        m = {"patches": patches[b], "w2": w2, "xyz_pm": xyz_pm,
             "xyz_wr": xyz_wr, "xyzb": xyzb_cm, "biases": biases,
             "ident": np.eye(128, dtype=np.float32).astype(bf)}
        m.update(wmats)
        in_maps.append(m)
    return in_maps, perms


def kernel(img_lr, xyz_hr, conv_w, conv_b,
           s1_w0, s1_b0, s1_w1, s1_b1, s1_w2, s1_b2, s1_w3, s1_b3,
           s2_w0, s2_b0, s2_w1, s2_b1, s2_w2, s2_b2, s2_w3, s2_b3,
           _trace=False):
    s1 = [(np.asarray(s1_w0), np.asarray(s1_b0)), (np.asarray(s1_w1), np.asarray(s1_b1)),
          (np.asarray(s1_w2), np.asarray(s1_b2)), (np.asarray(s1_w3), np.asarray(s1_b3))]
    s2 = [(np.asarray(s2_w0), np.asarray(s2_b0)), (np.asarray(s2_w1), np.asarray(s2_b1)),
          (np.asarray(s2_w2), np.asarray(s2_b2)), (np.asarray(s2_w3), np.asarray(s2_b3))]
    in_maps, perms = _prep_in_maps(np.asarray(img_lr), np.asarray(xyz_hr),
                                   np.asarray(conv_w), np.asarray(conv_b), s1, s2)
    nc = _get_compiled()
    res = run_bass_kernel_spmd(nc, in_maps, core_ids=list(range(8)), trace=_trace)
    out = np.zeros((N, K), np.float32)
    for core in range(8):
        b, q = core // 4, core % 4
        shard = np.empty(P, np.float32)
        shard[perms[core]] = res.results[core]["out"]
        out[b, q * (K // 4):q * (K // 4) + P] = shard
    kernel.last_exec_time_ns = res.exec_time_ns
    return out.reshape(N, 1, HWD, HWD, HWD)


kernel.last_exec_time_ns = None
